# revision 2
# baseline (speedup 1.0000x reference)
"""CAM (channel attention) module kernel for Trainium2, SPMD over 8 NeuronCores.

Reference computation (per batch b):
    V = x[b].reshape(C, N)                    # C=512, N=4096
    E = V @ V.T                               # C x C
    A = softmax(max_row(E) - E, axis=-1)      # == exp(min_row(E) - E) / rowsum
    out[b] = gamma * (A @ V) + x[b]

Sharding: data-parallel over batch. B=16 -> 2 batches per core.

v3 = baseline schedule shape + three changes:
  - Symmetric E: row tile ci's matmul only computes the upper blocks
    (cj >= ci, 10/16 of the contraction work). The lower blocks are
    reconstructed in-row: small fp32 SBUF copies of the off-diagonal
    upper blocks (made right after the source tile's matmul) are
    transposed by exact TensorE is_transpose (routing, bit-exact)
    directly into the row-tile PSUM bank, which stays one contiguous
    [128, 512] fp32 region for the fused softmax. Tiles processed
    ascending with softmax immediately per tile (banks freed fast,
    ps_e bufs=3).
  - bf16 stores: the epilogue STT writes bf16 and the output DMA moves
    half the bytes; host widens to fp32 (exact). Output values already
    carry bf16 residual rounding, so rel err stays ~2.5e-3.
  - xbar transposes split into k-halves so E(0) can start on the first
    half of the contraction while the second halves still stream.
"""

import numpy as np
from contextlib import ExitStack

import ml_dtypes

import concourse.bass as bass
import concourse.tile as tile
from concourse import bacc, mybir
from concourse.bass_utils import run_bass_kernel_spmd

B, C, HH, WW = 16, 512, 64, 64
N = HH * WW              # 4096
NCORES = 8
BPC = B // NCORES        # batches per core = 2

CT = C // 128            # 4 c-tiles
NK = N // 128            # 32 n-blocks (contraction chunks for E)

FP32 = mybir.dt.float32
BF16 = mybir.dt.bfloat16


def _build_kernel(reps=1):
    nc = bacc.Bacc(
        "TRN2",
        target_bir_lowering=False,
        debug=False,
        num_devices=NCORES,
    )

    x_ext = nc.dram_tensor("x", [BPC, C, N], FP32, kind="ExternalInput")
    g_ext = nc.dram_tensor("gamma", [1, 1], FP32, kind="ExternalInput")
    id_ext = nc.dram_tensor("ident", [128, 128], BF16, kind="ExternalInput")
    idf_ext = nc.dram_tensor("identf", [128, 128], FP32, kind="ExternalInput")
    out_ext = nc.dram_tensor("out", [BPC, C, N], BF16, kind="ExternalOutput")

    with tile.TileContext(nc) as tc:
        with ExitStack() as ctx:
            if reps == 0:
                _noop_body(ctx, tc, nc, g_ext, id_ext, out_ext)
            else:
                _body(ctx, tc, nc, x_ext, g_ext, id_ext, idf_ext, out_ext, reps)

    nc.compile()
    return nc


def _noop_body(ctx, tc, nc, g_ext, id_ext, out_ext):
    pool = ctx.enter_context(tc.tile_pool(name="np", bufs=1))
    t = pool.tile([1, 1], FP32, name="t")
    nc.sync.dma_start(t[:], g_ext[:, :])
    nc.gpsimd.dma_start(out_ext[0, 0:1, 0:1], t[:])


def _body(ctx, tc, nc, x_ext, g_ext, id_ext, idf_ext, out_ext, reps=1):
    consts = ctx.enter_context(tc.tile_pool(name="consts", bufs=1))
    xin_pool = ctx.enter_context(tc.tile_pool(name="xin", bufs=2))
    vn_pool = ctx.enter_context(tc.tile_pool(name="vn", bufs=2 * CT))
    vt_pool = ctx.enter_context(tc.tile_pool(name="vt", bufs=2))
    tx_pool = ctx.enter_context(tc.tile_pool(name="tx", bufs=CT))
    at_pool = ctx.enter_context(tc.tile_pool(name="at", bufs=2))
    st_pool = ctx.enter_context(tc.tile_pool(name="st", bufs=2 * CT))
    etr_pool = ctx.enter_context(tc.tile_pool(name="etr", bufs=2))
    out_pool = ctx.enter_context(tc.tile_pool(name="osb", bufs=2))

    ps_e = ctx.enter_context(tc.tile_pool(name="ps_e", bufs=3, space="PSUM"))
    ps_u = ctx.enter_context(tc.tile_pool(name="ps_u", bufs=3, space="PSUM"))
    ps_tr = ctx.enter_context(tc.tile_pool(name="ps_tr", bufs=2, space="PSUM"))

    ident = consts.tile([128, 128], BF16, name="ident")
    nc.sync.dma_start(ident[:], id_ext[:, :])
    identf = consts.tile([128, 128], FP32, name="identf")
    nc.sync.dma_start(identf[:], idf_ext[:, :])
    gam = consts.tile([1, 1], FP32, name="gam")
    nc.sync.dma_start(gam[:], g_ext[:, :])
    gbc = consts.tile([128, 1], FP32, name="gbc")
    nc.gpsimd.partition_broadcast(gbc[:], gam[:], channels=128)

    # per-pipeline-slot state
    state = {}

    def emit_load(b):
        vn = []
        for ct in range(CT):
            # HWDGE f32 load + ScalarE bf16 convert. All loads emitted
            # before any transpose: the SP ring is FIFO and a transpose's
            # semaphore wait would block later loads.
            xin = xin_pool.tile([128, N], FP32, name="xin", tag="xin")
            nc.sync.dma_start(xin[:], x_ext[b % BPC, ct * 128:(ct + 1) * 128, :])
            v = vn_pool.tile([128, N], BF16, name="vn", tag="vn")
            nc.scalar.copy(v[:], xin[:])
            vn.append(v)
        # vt[n_lo, ct, nb, c] = V[ct*128 + c, nb*128 + n_lo]
        vt = vt_pool.tile([128, CT, NK, 128], BF16, name="vt", tag="vt")
        for half in range(2):
            k0 = half * (NK // 2)
            for ct in range(1, CT):
                nc.sync.dma_start_transpose(
                    out=vt[:, ct, k0:k0 + NK // 2, :],
                    in_=vn[ct][:, k0 * 128:(k0 + NK // 2) * 128],
                )
        state[b] = (vn, vt)

    def emit_compute(b):
        vn, vt = state.pop(b)

        # ---- transpose the ct=0 slice on TensorE (out = V_blk.T @ I) ----
        for g in range(NK // 4):
            ps = ps_tr.tile([128, 512], FP32, name="ps_tr", tag="ps_tr")
            for i in range(4):
                nb = 4 * g + i
                nc.tensor.matmul(
                    ps[:, i * 128:(i + 1) * 128],
                    lhsT=vn[0][:, nb * 128:(nb + 1) * 128],
                    rhs=ident[:],
                    start=True,
                    stop=True,
                )
            dst = vt[:, 0, 4 * g:4 * g + 4, :]
            if g % 2 == 0:
                nc.vector.tensor_copy(dst, ps[:])
            else:
                nc.scalar.copy(dst, ps[:])

        def at_pe(ct):
            # at4[:, dj, ct, c] = tx[ct][c, dj*128 + d_lo]
            ps = ps_tr.tile([128, 512], FP32, name="ps_at", tag="ps_tr")
            for dj in range(CT):
                nc.tensor.matmul(
                    ps[:, dj * 128:(dj + 1) * 128],
                    lhsT=tx[ct][:, dj * 128:(dj + 1) * 128],
                    rhs=ident[:],
                    start=True,
                    stop=True,
                )
            nc.scalar.copy(at4[:, :, ct, :], ps[:])

        # ---- E rows (upper blocks only) + lower reconstruction + softmax ----
        tx = []
        rsg = []
        esb = etr_pool.tile([128, 6, 128], FP32, name="esb", tag="esb")
        # esb slot index for upper off-diagonal block (cj, ci), cj < ci
        eslot = {}
        slot = 0
        for cj in range(CT):
            for ci in range(cj + 1, CT):
                eslot[(cj, ci)] = slot
                slot += 1
        at4 = at_pool.tile([128, CT, CT, 128], BF16, name="at4", tag="at4")
        for ct in range(CT):
            pse = ps_e.tile([128, 512], FP32, name="ps_e", tag="ps_e")
            # upper blocks: E[ct-rows, ct*128:512)
            for k in range(NK):
                nc.tensor.matmul(
                    pse[:, ct * 128:],
                    lhsT=vt[:, ct, k, :],
                    rhs=vt[:, ct:, k, :],
                    start=(k == 0),
                    stop=(k == NK - 1),
                )
            # lower blocks: transpose earlier tiles' saved upper blocks
            for cj in range(ct):
                nc.tensor.matmul(
                    pse[:, cj * 128:(cj + 1) * 128],
                    lhsT=esb[:, eslot[(cj, ct)], :],
                    rhs=identf[:],
                    is_transpose=True,
                    start=True,
                    stop=True,
                )
            # save this tile's off-diagonal upper blocks for later tiles
            for ci in range(ct + 1, CT):
                nc.vector.tensor_copy(
                    esb[:, eslot[(ct, ci)], :],
                    pse[:, ci * 128:(ci + 1) * 128],
                )
            mmin = st_pool.tile([128, 1], FP32, name="mmin", tag="mmin")
            nc.vector.tensor_reduce(
                out=mmin[:], in_=pse[:],
                axis=mybir.AxisListType.X, op=mybir.AluOpType.min,
            )
            t = tx_pool.tile([128, 512], BF16, name="tx", tag="tx")
            ssum = st_pool.tile([128, 1], FP32, name="ssum", tag="ssum")
            # t = exp(min_row(E) - E), ssum = rowsum(t)
            nc.scalar.activation(
                t[:], pse[:], mybir.ActivationFunctionType.Exp,
                bias=mmin[:], scale=-1.0, accum_out=ssum[:],
            )
            rs = st_pool.tile([128, 1], FP32, name="rs", tag="rs")
            nc.vector.reciprocal(rs[:], ssum[:])
            rg = st_pool.tile([128, 1], FP32, name="rg", tag="rg")
            nc.vector.tensor_mul(rg[:], rs[:], gbc[:])   # gamma / S_c
            tx.append(t)
            rsg.append(rg)
            if ct >= 1:
                at_pe(ct - 1)
        at_pe(CT - 1)

        # ---- U = T @ V ; out = (gamma/S_c) * U + x, stored bf16 ----
        for ct in range(CT):
            for half in range(2):
                o = out_pool.tile([128, N // 2], BF16, name="osb", tag="osb")
                for nqh in range(4):
                    nq = half * 4 + nqh
                    psu = ps_u.tile([128, 512], FP32, name="ps_u", tag="ps_u")
                    for dj in range(CT):
                        nc.tensor.matmul(
                            psu[:],
                            lhsT=at4[:, dj, ct, :],
                            rhs=vn[dj][:, nq * 512:(nq + 1) * 512],
                            start=(dj == 0),
                            stop=(dj == CT - 1),
                        )
                    nc.vector.scalar_tensor_tensor(
                        out=o[:, nqh * 512:(nqh + 1) * 512],
                        in0=psu[:],
                        scalar=rsg[ct][:],
                        in1=vn[ct][:, nq * 512:(nq + 1) * 512],
                        op0=mybir.AluOpType.mult,
                        op1=mybir.AluOpType.add,
                    )
                nc.scalar.dma_start(
                    out_ext[
                        b % BPC,
                        ct * 128:(ct + 1) * 128,
                        half * (N // 2):(half + 1) * (N // 2),
                    ],
                    o[:],
                )

    nb_total = reps * BPC
    prefetch = 1
    emit_load(0)
    for j in range(1, min(prefetch, nb_total)):
        emit_load(j)
    for i in range(nb_total):
        if i + prefetch < nb_total:
            emit_load(i + prefetch)
        emit_compute(i)


_NC_CACHE = {}


def _get_nc(reps=1):
    if reps not in _NC_CACHE:
        _NC_CACHE[reps] = _build_kernel(reps)
    return _NC_CACHE[reps]


def kernel(x: np.ndarray, gamma: np.ndarray) -> np.ndarray:
    assert x.shape == (B, C, HH, WW), x.shape
    nc = _get_nc()

    xr = np.ascontiguousarray(x, dtype=np.float32).reshape(B, C, N)
    g2 = np.asarray(gamma, dtype=np.float32).reshape(1, 1)
    ident = np.eye(128, dtype=ml_dtypes.bfloat16)
    identf = np.eye(128, dtype=np.float32)

    in_maps = []
    for i in range(NCORES):
        in_maps.append({
            "x": xr[i * BPC:(i + 1) * BPC],
            "gamma": g2,
            "ident": ident,
            "identf": identf,
        })

    res = run_bass_kernel_spmd(nc, in_maps, core_ids=list(range(NCORES)))
    outs = [res.results[i]["out"] for i in range(NCORES)]
    full = np.concatenate(outs, axis=0).reshape(B, C, HH, WW)
    return full.astype(np.float32)
